# revision 23
# baseline (speedup 1.0000x reference)
"""Trainium2 Bass kernel for nn_EnsembleSpace (moe_routing).

Reference computation (B=128, E=64, D1=512, D2=2048):
    idx  = top_k(config, 8)                     # [B, E] routing logits
    cfg  = softmax(config * topk_mask)          # full-width softmax
    cfg  = where(cfg < 1e-4, 0, cfg)
    out  = cfg @ kernel.reshape(E, D1*D2)       # [B, D1*D2] -> [B, D1, D2]

Sharding: the big operands are the expert table (read once) and the
output (written once).  Sharding the *feature* axis (D1) over the 8
cores means each core reads 1/8 of the table and writes 1/8 of the
output with no collective at all.  (E-sharding per the hint would need
a 512 MB all-reduce; B-sharding would read the full table on every
core.)

Precision: the correctness gate is rel_err < 2e-2 against the f32
reference; an fp16 table + fp16 routing weights + f32 PSUM accumulate
+ fp16 output lands at ~6e-4 (measured on the exact problem data).
Running the whole stream in fp16 halves the HBM traffic — per core
16 MB table read + 32 MB output write = 48 MB vs 96 MB for f32.

Each core:
  1. computes the routing weights cfg [128, 64] on-chip in f32
     (iterative top-8 via 7 max+knockout rounds, exp+sum via one ACT
     op, eps mask),
  2. transposes cfg to [E, B] via two col-tiled identity matmuls so the
     weights land in BOTH partition halves (rows 0-63 and 64-127),
     downcast to fp16,
  3. streams its table slice as 16 chunk-QUADS of [128, 4096] fp16
     (1 MB DMAs, 8 KB/partition); each quad runs as 2x (2x4) row-packed
     fp16 matmuls (K=64 tiles at array rows 0-63 / 64-127, concurrent),
     PSUM->SBUF downcast copies split across DVE and ACT, 1 MB fp16 out
     DMAs per chunk-pair.

Input DMAs ride the SP HWDGE ring, output DMAs the ACT ring, so the two
streams don't serialize on one descriptor FIFO.  The host downcasts the
table to fp16 while re-tiling it and upcasts the result.
"""

import sys

for _p in ("/opt/trn_rl_repo", "/root/.axon_site/_ro/trn_rl_repo"):
    if _p not in sys.path:
        sys.path.append(_p)

import numpy as np
import concourse.bass as bass
from concourse import tile, masks, bass_utils

mybir = bass.mybir
_f32 = mybir.dt.float32
_f16 = mybir.dt.float16
_X = mybir.AxisListType.X
_alu = mybir.AluOpType

B, E, D1, D2 = 128, 64, 512, 2048
N_CORES = 8
D1_SH = D1 // N_CORES          # 64 D1-rows (chunks) per core
CH = D2                        # chunk free size (2048 f16 = 4 KB/partition)
MM_N = 512                     # one matmul / PSUM bank
N_MM = CH // MM_N
ROWS_IN = 8                    # D1-rows per input tile  (2 MB fp16 DMAs)
ROWS_OUT = 4                   # D1-rows per output tile (2 MB fp16 DMAs)
NT = D1_SH // ROWS_IN          # 8 input tiles per core
TOP_K = 8
SPARSE_EPS = 1e-4

_TRACE = False                 # test.py flips this for profiled runs
_TRACE_KWARGS = {}
LAST_RESULT = None             # BassKernelResults of the last run


def _split_multi_waits(nc):
    """This walrus build rejects >1 sync-wait per instruction.  Tile's
    add_semaphores emits multi-wait instructions (and the kernel-tail drain
    waits on every live semaphore).  Move the extra waits onto same-engine
    nops inserted immediately before the instruction — the engine executes
    serially, so blocking on the nops is equivalent."""
    n_split = 0
    for bb in nc.m.functions[0].blocks:
        out = []
        changed = False
        for inst in bb.instructions:
            si = inst.sync_info
            waits = list(si.on_wait) if (si is not None and si.on_wait) else []
            if len(waits) > 1:
                changed = True
                for w in waits[:-1]:
                    n_split += 1
                    nop = mybir.InstNoOp(name=f"I-waitsplit-{n_split}")
                    nop.engine = inst.engine
                    nop.sync_info = mybir.SyncInfo(on_wait=[w], on_update=[])
                    out.append(nop)
                inst.sync_info = mybir.SyncInfo(
                    on_wait=[waits[-1]], on_update=list(si.on_update or [])
                )
            out.append(inst)
        if changed:
            bb.instructions = out


def _routing_weights(nc, rp, pp, cfgin):
    """cfgin [B, E] f32 -> cfgT [E, B] fp16 in SBUF (top-8, softmax, eps)."""
    # 8th-largest per row, in exp-space: exp(config) is positive and
    # order-preserving, so "knock out the max" is a 2-op zero-replace
    # (zero can never shadow a remaining value) instead of a 3-op -inf add
    e0 = rp.tile([B, E], _f32, tag="e0")
    nc.scalar.activation(e0[:], cfgin[:], mybir.ActivationFunctionType.Exp)
    t = rp.tile([B, E], _f32, tag="t")
    nc.vector.tensor_copy(t[:], e0[:])
    mk = rp.tile([B, 1], _f32, tag="mk")
    for _ in range(TOP_K - 1):
        nc.vector.reduce_max(mk[:], t[:], axis=_X)
        nc.vector.scalar_tensor_tensor(
            t[:], t[:], mk[:], t[:], op0=_alu.is_lt, op1=_alu.mult
        )
    m8 = rp.tile([B, 1], _f32, tag="m8")
    nc.vector.reduce_max(m8[:], t[:], axis=_X)

    # cfg0 = (exp(config) >= exp(m8)) * config ; softmax ; eps mask
    cfg0 = rp.tile([B, E], _f32, tag="cfg0")
    nc.vector.scalar_tensor_tensor(
        cfg0[:], e0[:], m8[:], cfgin[:], op0=_alu.is_ge, op1=_alu.mult
    )
    ecfg = rp.tile([B, E], _f32, tag="ecfg")
    zs = rp.tile([B, 1], _f32, tag="zs")
    nc.scalar.activation(
        ecfg[:], cfg0[:], mybir.ActivationFunctionType.Exp, accum_out=zs[:]
    )
    rz = rp.tile([B, 1], _f32, tag="rz")
    nc.vector.reciprocal(rz[:], zs[:])
    cfgn = rp.tile([B, E], _f32, tag="cfgn")
    nc.vector.tensor_scalar_mul(cfgn[:], ecfg[:], rz[:])
    cfgf = rp.tile([B, E], _f32, tag="cfgf")
    nc.vector.scalar_tensor_tensor(
        cfgf[:], cfgn[:], SPARSE_EPS, cfgn[:], op0=_alu.is_ge, op1=_alu.mult
    )

    # transpose to [E, B], replicated into both partition halves so the
    # row-packed matmuls can source weights at array rows 0-63 and 64-127
    ident = rp.tile([B, B], _f32, tag="ident")
    masks.make_identity(nc, ident[:])
    psT = pp.tile([B, B], _f32, tag="ps")
    nc.tensor.matmul(psT[0:E, :], cfgf[:], ident[:], start=True, stop=True)
    nc.tensor.matmul(psT[E:2 * E, :], cfgf[:], ident[:], start=True, stop=True)
    cfgT2 = rp.tile([B, B], _f16, tag="cfgT2")
    nc.vector.tensor_copy(cfgT2[:], psT[:])
    return cfgT2


def _build():
    nc = bass.Bass(
        "TRN2", target_bir_lowering=False, debug=False, num_devices=N_CORES
    )
    cfg_ap = nc.dram_tensor("config", [B, E], _f32, kind="ExternalInput").ap()
    ks_ap = nc.dram_tensor(
        "kslice", [NT, 2 * E, 2 * E + (ROWS_IN // 2) * CH], _f16,
        kind="ExternalInput",
    ).ap()
    out_ap = nc.dram_tensor(
        "out", [D1_SH // ROWS_OUT, B, ROWS_OUT * CH], _f16,
        kind="ExternalOutput",
    ).ap()

    with tile.TileContext(nc) as tc:
        with tc.tile_pool(name="route", bufs=1) as rp, \
             tc.tile_pool(name="inp", bufs=3) as ip, \
             tc.tile_pool(name="outp", bufs=6) as op_, \
             tc.tile_pool(name="ps", bufs=8, space="PSUM") as pp:
            # the f32 config is bitcast-embedded as 128 f16 columns at
            # the head of every table tile, so there is NO separate config
            # DMA: the table stream owns the SP ring from preamble-end (a
            # 32 KB config DMA ahead of it costs ~6 us of ring
            # serialization; on the ACT ring it starves ~12 us behind the
            # table descriptor trains; SWDGE takes ~12 us to spin up).
            kts = []
            for t in range(2):
                kt = ip.tile(
                    [2 * E, 2 * E + (ROWS_IN // 2) * CH], _f16, tag="kt"
                )
                nc.sync.dma_start(kt[:], ks_ap[t])
                kts.append(kt)
            cfgin = kts[0][:, 0:2 * E].bitcast(_f32)
            cfgT2 = _routing_weights(nc, rp, pp, cfgin)
            n_out = 0
            for t in range(NT):
                if t < 2:
                    kt = kts[t]
                else:
                    kt = ip.tile(
                        [2 * E, 2 * E + (ROWS_IN // 2) * CH], _f16, tag="kt"
                    )
                    nc.sync.dma_start(kt[:], ks_ap[t])
                for half in range(ROWS_IN // ROWS_OUT):
                    # output tile covers D1-rows 8t+4*half .. +3, as
                    # ROWS_OUT col-blocks of CH; input pair c (cols c*CH)
                    # holds rows (8t+2c, 8t+2c+1) in partition halves
                    ot = op_.tile([B, ROWS_OUT * CH], _f16, tag="ot")
                    for cc in range(ROWS_OUT // 2):
                        c = half * (ROWS_OUT // 2) + cc
                        for j in range(N_MM):
                            js_in = slice(2 * E + c * CH + j * MM_N,
                                          2 * E + c * CH + (j + 1) * MM_N)
                            jsA = slice(2 * cc * CH + j * MM_N,
                                        2 * cc * CH + (j + 1) * MM_N)
                            jsB = slice((2 * cc + 1) * CH + j * MM_N,
                                        (2 * cc + 1) * CH + (j + 1) * MM_N)
                            psA = pp.tile([B, MM_N], _f32, tag="ps")
                            nc.tensor.matmul(
                                psA[:], cfgT2[0:E, :], kt[0:E, js_in],
                                start=True, stop=True,
                            )
                            psB = pp.tile([B, MM_N], _f32, tag="ps")
                            nc.tensor.matmul(
                                psB[:], cfgT2[E:2 * E, :], kt[E:2 * E, js_in],
                                start=True, stop=True,
                            )
                            if j % 2 == 0:
                                nc.vector.tensor_copy(ot[:, jsA], psA[:])
                                nc.scalar.copy(ot[:, jsB], psB[:])
                            else:
                                nc.scalar.copy(ot[:, jsA], psA[:])
                                nc.vector.tensor_copy(ot[:, jsB], psB[:])
                    # alternate output DMAs across both HWDGE rings so the
                    # write stream can saturate HBM during the drain phase
                    eng = nc.scalar if n_out % 2 == 0 else nc.sync
                    n_out += 1
                    eng.dma_start(out_ap[2 * t + half], ot[:])
    _split_multi_waits(nc)
    return nc


_NC_CACHE = None


def _get_nc():
    global _NC_CACHE
    if _NC_CACHE is None:
        _NC_CACHE = _build()
    return _NC_CACHE


def kernel(config, kernel):
    global LAST_RESULT
    config = np.ascontiguousarray(np.asarray(config, dtype=np.float32))
    ktab = np.asarray(kernel, dtype=np.float32).reshape(E, D1, D2)

    in_maps = []
    for c in range(N_CORES):
        # this core's D1 rows, chunk-major [D1_SH, E, D2] fp16, regrouped
        # into 8 tiles [128, 4*D2]: partition i = (h*64+e) holds expert e
        # of D1-row 8t+2c2+h at free columns c2*D2..c2*D2+D2.
        ksl = ktab[:, c * D1_SH:(c + 1) * D1_SH, :].transpose(1, 0, 2)
        ksl = ksl.reshape(NT, ROWS_IN // 2, 2, E, D2).transpose(0, 2, 3, 1, 4)
        ksl = np.asarray(ksl, dtype=np.float16).reshape(
            NT, 2 * E, (ROWS_IN // 2) * D2
        )
        cfg16 = np.broadcast_to(
            config.view(np.float16)[None], (NT, 2 * E, 2 * E)
        )
        ksl = np.ascontiguousarray(np.concatenate([cfg16, ksl], axis=2))
        in_maps.append({"config": config, "kslice": ksl})

    nc = _get_nc()
    res = bass_utils.run_bass_kernel_spmd(
        nc,
        in_maps,
        list(range(N_CORES)),
        trace=_TRACE,
        **_TRACE_KWARGS,
    )
    LAST_RESULT = res

    out = np.empty((B, D1, D2), dtype=np.float32)
    for c in range(N_CORES):
        # out DRAM [16, B, 4*D2] fp16: row r holds D1-rows 4r..4r+3 as
        # 4 col-blocks of D2
        o = res.results[c]["out"].reshape(D1_SH // ROWS_OUT, B, ROWS_OUT, D2)
        o = o.transpose(0, 2, 1, 3).reshape(D1_SH, B, D2)
        out[:, c * D1_SH:(c + 1) * D1_SH, :] = o.transpose(1, 0, 2)
    return out


# revision 24
# speedup vs baseline: 1.0774x; 1.0774x over previous
"""Trainium2 Bass kernel for nn_EnsembleSpace (moe_routing).

Reference computation (B=128, E=64, D1=512, D2=2048):
    idx  = top_k(config, 8)                     # [B, E] routing logits
    cfg  = softmax(config * topk_mask)          # full-width softmax
    cfg  = where(cfg < 1e-4, 0, cfg)
    out  = cfg @ kernel.reshape(E, D1*D2)       # [B, D1*D2] -> [B, D1, D2]

Sharding: the big operands are the expert table (read once) and the
output (written once).  Sharding the *feature* axis (D1) over the 8
cores means each core reads 1/8 of the table and writes 1/8 of the
output with no collective at all.  (E-sharding per the hint would need
a 512 MB all-reduce; B-sharding would read the full table on every
core.)

Precision: the correctness gate is rel_err < 2e-2 against the f32
reference; an fp16 table + fp16 routing weights + f32 PSUM accumulate
+ fp16 output lands at ~6e-4 (measured on the exact problem data).
Running the whole stream in fp16 halves the HBM traffic — per core
16 MB table read + 32 MB output write = 48 MB vs 96 MB for f32.

Each core:
  1. computes the routing weights cfg [128, 64] on-chip in f32
     (iterative top-8 via 7 max+knockout rounds, exp+sum via one ACT
     op, eps mask),
  2. transposes cfg to [E, B] via two col-tiled identity matmuls so the
     weights land in BOTH partition halves (rows 0-63 and 64-127),
     downcast to fp16,
  3. streams its table slice as 8 tiles of [128, 128+8192] fp16 (2 MB
     DMAs; the first 128 columns are the f32 config bitcast to f16, so
     no separate config DMA delays either HWDGE ring); each tile runs
     as 16 pairs of row-packed fp16 matmuls (K=64 at array rows 0-63 /
     64-127, concurrent), PSUM->SBUF downcast copies split across DVE
     and ACT into [128, 8192] fp16 output tiles, 2 MB out DMAs.

Input DMAs ride the SP HWDGE ring; output DMAs alternate between the
SP and ACT rings so the write stream can saturate HBM during the final
output-only drain.  The outp pool is 6 tiles deep so the produced-but-
not-drained backlog bridges production gaps.  The host downcasts the
table to fp16 while re-tiling it and upcasts the fp16 result to f32.
"""

import sys

for _p in ("/opt/trn_rl_repo", "/root/.axon_site/_ro/trn_rl_repo"):
    if _p not in sys.path:
        sys.path.append(_p)

import numpy as np
import concourse.bass as bass
from concourse import tile, masks, bass_utils

mybir = bass.mybir
_f32 = mybir.dt.float32
_f16 = mybir.dt.float16
_X = mybir.AxisListType.X
_alu = mybir.AluOpType

B, E, D1, D2 = 128, 64, 512, 2048
N_CORES = 8
D1_SH = D1 // N_CORES          # 64 D1-rows (chunks) per core
CH = D2                        # chunk free size (2048 f16 = 4 KB/partition)
MM_N = 512                     # one matmul / PSUM bank
N_MM = CH // MM_N
ROWS_IN = 8                    # D1-rows per input tile  (2 MB fp16 DMAs)
ROWS_OUT = 4                   # D1-rows per output tile (2 MB fp16 DMAs)
NT = D1_SH // ROWS_IN          # 8 input tiles per core
TOP_K = 8
SPARSE_EPS = 1e-4

_TRACE = False                 # test.py flips this for profiled runs
_TRACE_KWARGS = {}
LAST_RESULT = None             # BassKernelResults of the last run


def _split_multi_waits(nc):
    """This walrus build rejects >1 sync-wait per instruction.  Tile's
    add_semaphores emits multi-wait instructions (and the kernel-tail drain
    waits on every live semaphore).  Move the extra waits onto same-engine
    nops inserted immediately before the instruction — the engine executes
    serially, so blocking on the nops is equivalent."""
    n_split = 0
    for bb in nc.m.functions[0].blocks:
        out = []
        changed = False
        for inst in bb.instructions:
            si = inst.sync_info
            waits = list(si.on_wait) if (si is not None and si.on_wait) else []
            if len(waits) > 1:
                changed = True
                for w in waits[:-1]:
                    n_split += 1
                    nop = mybir.InstNoOp(name=f"I-waitsplit-{n_split}")
                    nop.engine = inst.engine
                    nop.sync_info = mybir.SyncInfo(on_wait=[w], on_update=[])
                    out.append(nop)
                inst.sync_info = mybir.SyncInfo(
                    on_wait=[waits[-1]], on_update=list(si.on_update or [])
                )
            out.append(inst)
        if changed:
            bb.instructions = out


def _routing_weights(nc, rp, pp, cfgin):
    """cfgin [B, E] f32 -> cfgT [E, B] fp16 in SBUF (top-8, softmax, eps)."""
    # 8th-largest per row, in exp-space: exp(config) is positive and
    # order-preserving, so "knock out the max" is a 2-op zero-replace
    # (zero can never shadow a remaining value) instead of a 3-op -inf add
    e0 = rp.tile([B, E], _f32, tag="e0")
    nc.scalar.activation(e0[:], cfgin[:], mybir.ActivationFunctionType.Exp)
    t = rp.tile([B, E], _f32, tag="t")
    nc.vector.tensor_copy(t[:], e0[:])
    mk = rp.tile([B, 1], _f32, tag="mk")
    for _ in range(TOP_K - 1):
        nc.vector.reduce_max(mk[:], t[:], axis=_X)
        nc.vector.scalar_tensor_tensor(
            t[:], t[:], mk[:], t[:], op0=_alu.is_lt, op1=_alu.mult
        )
    m8 = rp.tile([B, 1], _f32, tag="m8")
    nc.vector.reduce_max(m8[:], t[:], axis=_X)

    # cfg0 = (exp(config) >= exp(m8)) * config ; softmax ; eps mask
    cfg0 = rp.tile([B, E], _f32, tag="cfg0")
    nc.vector.scalar_tensor_tensor(
        cfg0[:], e0[:], m8[:], cfgin[:], op0=_alu.is_ge, op1=_alu.mult
    )
    ecfg = rp.tile([B, E], _f32, tag="ecfg")
    zs = rp.tile([B, 1], _f32, tag="zs")
    nc.scalar.activation(
        ecfg[:], cfg0[:], mybir.ActivationFunctionType.Exp, accum_out=zs[:]
    )
    rz = rp.tile([B, 1], _f32, tag="rz")
    nc.vector.reciprocal(rz[:], zs[:])
    cfgn = rp.tile([B, E], _f32, tag="cfgn")
    nc.vector.tensor_scalar_mul(cfgn[:], ecfg[:], rz[:])
    cfgf = rp.tile([B, E], _f32, tag="cfgf")
    nc.vector.scalar_tensor_tensor(
        cfgf[:], cfgn[:], SPARSE_EPS, cfgn[:], op0=_alu.is_ge, op1=_alu.mult
    )

    # transpose to [E, B], replicated into both partition halves so the
    # row-packed matmuls can source weights at array rows 0-63 and 64-127
    ident = rp.tile([B, B], _f32, tag="ident")
    masks.make_identity(nc, ident[:])
    psT = pp.tile([B, B], _f32, tag="ps")
    nc.tensor.matmul(psT[0:E, :], cfgf[:], ident[:], start=True, stop=True)
    nc.tensor.matmul(psT[E:2 * E, :], cfgf[:], ident[:], start=True, stop=True)
    cfgT2 = rp.tile([B, B], _f16, tag="cfgT2")
    nc.vector.tensor_copy(cfgT2[:], psT[:])
    return cfgT2


def _build():
    nc = bass.Bass(
        "TRN2", target_bir_lowering=False, debug=False, num_devices=N_CORES
    )
    cfg_ap = nc.dram_tensor("config", [B, E], _f32, kind="ExternalInput").ap()
    ks_ap = nc.dram_tensor(
        "kslice", [NT, 2 * E, 2 * E + (ROWS_IN // 2) * CH], _f16,
        kind="ExternalInput",
    ).ap()
    out_ap = nc.dram_tensor(
        "out", [D1_SH // ROWS_OUT, B, ROWS_OUT * CH], _f16,
        kind="ExternalOutput",
    ).ap()

    with tile.TileContext(nc) as tc:
        with tc.tile_pool(name="route", bufs=1) as rp, \
             tc.tile_pool(name="inp", bufs=3) as ip, \
             tc.tile_pool(name="outp", bufs=6) as op_, \
             tc.tile_pool(name="ps", bufs=8, space="PSUM") as pp:
            # the f32 config is bitcast-embedded as 128 f16 columns at
            # the head of every table tile, so there is NO separate config
            # DMA: the table stream owns the SP ring from preamble-end (a
            # 32 KB config DMA ahead of it costs ~6 us of ring
            # serialization; on the ACT ring it starves ~12 us behind the
            # table descriptor trains; SWDGE takes ~12 us to spin up).
            kts = []
            for t in range(2):
                kt = ip.tile(
                    [2 * E, 2 * E + (ROWS_IN // 2) * CH], _f16, tag="kt"
                )
                if t == 0:
                    # split tile 0 into a small head DMA (config + first
                    # chunk-pair, ~560 KB) and the remainder: the routing
                    # chain unblocks at ~5 us instead of waiting for the
                    # full 2 MB, so the matmul pipeline starts ~10 us
                    # earlier and the input stream never starves behind
                    # a compute crunch.  (Range-based deps: c=0 matmuls
                    # only wait on the head DMA.)
                    hd = 2 * E + CH
                    nc.sync.dma_start(kt[:, 0:hd], ks_ap[t][:, 0:hd])
                    nc.sync.dma_start(kt[:, hd:], ks_ap[t][:, hd:])
                else:
                    nc.sync.dma_start(kt[:], ks_ap[t])
                kts.append(kt)
            cfgin = kts[0][:, 0:2 * E].bitcast(_f32)
            cfgT2 = _routing_weights(nc, rp, pp, cfgin)
            n_out = 0
            for t in range(NT):
                if t < 2:
                    kt = kts[t]
                else:
                    kt = ip.tile(
                        [2 * E, 2 * E + (ROWS_IN // 2) * CH], _f16, tag="kt"
                    )
                    if t == NT - 1:
                        # split the last tile too: its first half's
                        # matmuls start ~3 us earlier, shortening the
                        # output-only drain at the end
                        md = 2 * E + 2 * CH
                        nc.sync.dma_start(kt[:, 0:md], ks_ap[t][:, 0:md])
                        nc.sync.dma_start(kt[:, md:], ks_ap[t][:, md:])
                    else:
                        nc.sync.dma_start(kt[:], ks_ap[t])
                for half in range(ROWS_IN // ROWS_OUT):
                    # output tile covers D1-rows 8t+4*half .. +3, as
                    # ROWS_OUT col-blocks of CH; input pair c (cols c*CH)
                    # holds rows (8t+2c, 8t+2c+1) in partition halves
                    ot = op_.tile([B, ROWS_OUT * CH], _f16, tag="ot")
                    for cc in range(ROWS_OUT // 2):
                        c = half * (ROWS_OUT // 2) + cc
                        for j in range(N_MM):
                            js_in = slice(2 * E + c * CH + j * MM_N,
                                          2 * E + c * CH + (j + 1) * MM_N)
                            jsA = slice(2 * cc * CH + j * MM_N,
                                        2 * cc * CH + (j + 1) * MM_N)
                            jsB = slice((2 * cc + 1) * CH + j * MM_N,
                                        (2 * cc + 1) * CH + (j + 1) * MM_N)
                            psA = pp.tile([B, MM_N], _f32, tag="ps")
                            nc.tensor.matmul(
                                psA[:], cfgT2[0:E, :], kt[0:E, js_in],
                                start=True, stop=True,
                            )
                            psB = pp.tile([B, MM_N], _f32, tag="ps")
                            nc.tensor.matmul(
                                psB[:], cfgT2[E:2 * E, :], kt[E:2 * E, js_in],
                                start=True, stop=True,
                            )
                            if j % 2 == 0:
                                nc.vector.tensor_copy(ot[:, jsA], psA[:])
                                nc.scalar.copy(ot[:, jsB], psB[:])
                            else:
                                nc.scalar.copy(ot[:, jsA], psA[:])
                                nc.vector.tensor_copy(ot[:, jsB], psB[:])
                    # alternate output DMAs across both HWDGE rings so the
                    # write stream can saturate HBM during the drain phase
                    eng = nc.scalar if n_out % 2 == 0 else nc.sync
                    n_out += 1
                    eng.dma_start(out_ap[2 * t + half], ot[:])
    _split_multi_waits(nc)
    return nc


_NC_CACHE = None


def _get_nc():
    global _NC_CACHE
    if _NC_CACHE is None:
        _NC_CACHE = _build()
    return _NC_CACHE


def kernel(config, kernel):
    global LAST_RESULT
    config = np.ascontiguousarray(np.asarray(config, dtype=np.float32))
    ktab = np.asarray(kernel, dtype=np.float32).reshape(E, D1, D2)

    in_maps = []
    for c in range(N_CORES):
        # this core's D1 rows, chunk-major [D1_SH, E, D2] fp16, regrouped
        # into 8 tiles [128, 4*D2]: partition i = (h*64+e) holds expert e
        # of D1-row 8t+2c2+h at free columns c2*D2..c2*D2+D2.
        ksl = ktab[:, c * D1_SH:(c + 1) * D1_SH, :].transpose(1, 0, 2)
        ksl = ksl.reshape(NT, ROWS_IN // 2, 2, E, D2).transpose(0, 2, 3, 1, 4)
        ksl = np.asarray(ksl, dtype=np.float16).reshape(
            NT, 2 * E, (ROWS_IN // 2) * D2
        )
        cfg16 = np.broadcast_to(
            config.view(np.float16)[None], (NT, 2 * E, 2 * E)
        )
        ksl = np.ascontiguousarray(np.concatenate([cfg16, ksl], axis=2))
        in_maps.append({"config": config, "kslice": ksl})

    nc = _get_nc()
    res = bass_utils.run_bass_kernel_spmd(
        nc,
        in_maps,
        list(range(N_CORES)),
        trace=_TRACE,
        **_TRACE_KWARGS,
    )
    LAST_RESULT = res

    out = np.empty((B, D1, D2), dtype=np.float32)
    for c in range(N_CORES):
        # out DRAM [16, B, 4*D2] fp16: row r holds D1-rows 4r..4r+3 as
        # 4 col-blocks of D2
        o = res.results[c]["out"].reshape(D1_SH // ROWS_OUT, B, ROWS_OUT, D2)
        o = o.transpose(0, 2, 1, 3).reshape(D1_SH, B, D2)
        out[:, c * D1_SH:(c + 1) * D1_SH, :] = o.transpose(1, 0, 2)
    return out


# revision 25
# speedup vs baseline: 1.2101x; 1.1232x over previous
"""Trainium2 Bass kernel for nn_EnsembleSpace (moe_routing).

Reference computation (B=128, E=64, D1=512, D2=2048):
    idx  = top_k(config, 8)                     # [B, E] routing logits
    cfg  = softmax(config * topk_mask)          # full-width softmax
    cfg  = where(cfg < 1e-4, 0, cfg)
    out  = cfg @ kernel.reshape(E, D1*D2)       # [B, D1*D2] -> [B, D1, D2]

Sharding: the big operands are the expert table (read once) and the
output (written once).  Sharding the *feature* axis (D1) over the 8
cores means each core reads 1/8 of the table and writes 1/8 of the
output with no collective at all.  (E-sharding per the hint would need
a 512 MB all-reduce; B-sharding would read the full table on every
core.)

Precision: the correctness gate is rel_err < 2e-2 against the f32
reference; an fp16 table + fp16 routing weights + f32 PSUM accumulate
+ fp16 output lands at ~6e-4 (measured on the exact problem data).
Running the whole stream in fp16 halves the HBM traffic — per core
16 MB table read + 32 MB output write = 48 MB vs 96 MB for f32.

Each core:
  1. computes the routing weights cfg [128, 64] on-chip in f32
     (iterative top-8 via 7 max+knockout rounds, exp+sum via one ACT
     op, eps mask),
  2. transposes cfg to [E, B] via two col-tiled identity matmuls so the
     weights land in BOTH partition halves (rows 0-63 and 64-127),
     downcast to fp16,
  3. streams its table slice as 8 tiles of [128, 128+8192] fp16 (2 MB
     DMAs; the first 128 columns are the f32 config bitcast to f16, so
     no separate config DMA delays either HWDGE ring); each tile runs
     as 16 pairs of row-packed fp16 matmuls (K=64 at array rows 0-63 /
     64-127, concurrent), PSUM->SBUF downcast copies split across DVE
     and ACT into [128, 8192] fp16 output tiles, 2 MB out DMAs.

Input DMAs ride the SP HWDGE ring; output DMAs alternate between the
SP and ACT rings so the write stream can saturate HBM during the final
output-only drain.  The outp pool is 6 tiles deep so the produced-but-
not-drained backlog bridges production gaps.  The host downcasts the
table to fp16 while re-tiling it and upcasts the fp16 result to f32.
"""

import sys

for _p in ("/opt/trn_rl_repo", "/root/.axon_site/_ro/trn_rl_repo"):
    if _p not in sys.path:
        sys.path.append(_p)

import numpy as np
import concourse.bass as bass
from concourse import tile, masks, bass_utils

mybir = bass.mybir
_f32 = mybir.dt.float32
_f16 = mybir.dt.float16
_X = mybir.AxisListType.X
_alu = mybir.AluOpType

B, E, D1, D2 = 128, 64, 512, 2048
N_CORES = 8
D1_SH = D1 // N_CORES          # 64 D1-rows (chunks) per core
CH = D2                        # chunk free size (2048 f16 = 4 KB/partition)
MM_N = 512                     # one matmul / PSUM bank
N_MM = CH // MM_N
ROWS_IN = 8                    # D1-rows per input tile  (2 MB fp16 DMAs)
ROWS_OUT = 4                   # D1-rows per output tile (2 MB fp16 DMAs)
NT = D1_SH // ROWS_IN          # 8 input tiles per core
TOP_K = 8
SPARSE_EPS = 1e-4

_TRACE = False                 # test.py flips this for profiled runs
_TRACE_KWARGS = {}
LAST_RESULT = None             # BassKernelResults of the last run


def _split_multi_waits(nc):
    """This walrus build rejects >1 sync-wait per instruction.  Tile's
    add_semaphores emits multi-wait instructions (and the kernel-tail drain
    waits on every live semaphore).  Move the extra waits onto same-engine
    nops inserted immediately before the instruction — the engine executes
    serially, so blocking on the nops is equivalent."""
    n_split = 0
    for bb in nc.m.functions[0].blocks:
        out = []
        changed = False
        for inst in bb.instructions:
            si = inst.sync_info
            waits = list(si.on_wait) if (si is not None and si.on_wait) else []
            if len(waits) > 1:
                changed = True
                for w in waits[:-1]:
                    n_split += 1
                    nop = mybir.InstNoOp(name=f"I-waitsplit-{n_split}")
                    nop.engine = inst.engine
                    nop.sync_info = mybir.SyncInfo(on_wait=[w], on_update=[])
                    out.append(nop)
                inst.sync_info = mybir.SyncInfo(
                    on_wait=[waits[-1]], on_update=list(si.on_update or [])
                )
            out.append(inst)
        if changed:
            bb.instructions = out


def _routing_weights(nc, rp, pp, cfgin):
    """cfgin [B, E] f32 -> cfgT [E, B] fp16 in SBUF (top-8, softmax, eps)."""
    # 8th-largest per row, in exp-space: exp(config) is positive and
    # order-preserving, so "knock out the max" is a 2-op zero-replace
    # (zero can never shadow a remaining value) instead of a 3-op -inf add
    e0 = rp.tile([B, E], _f32, tag="e0")
    nc.scalar.activation(e0[:], cfgin[:], mybir.ActivationFunctionType.Exp)
    t = rp.tile([B, E], _f32, tag="t")
    nc.vector.tensor_copy(t[:], e0[:])
    mk = rp.tile([B, 1], _f32, tag="mk")
    for _ in range(TOP_K - 1):
        nc.vector.reduce_max(mk[:], t[:], axis=_X)
        nc.vector.scalar_tensor_tensor(
            t[:], t[:], mk[:], t[:], op0=_alu.is_lt, op1=_alu.mult
        )
    m8 = rp.tile([B, 1], _f32, tag="m8")
    nc.vector.reduce_max(m8[:], t[:], axis=_X)

    # cfg0 = (exp(config) >= exp(m8)) * config ; softmax ; eps mask
    cfg0 = rp.tile([B, E], _f32, tag="cfg0")
    nc.vector.scalar_tensor_tensor(
        cfg0[:], e0[:], m8[:], cfgin[:], op0=_alu.is_ge, op1=_alu.mult
    )
    ecfg = rp.tile([B, E], _f32, tag="ecfg")
    zs = rp.tile([B, 1], _f32, tag="zs")
    nc.scalar.activation(
        ecfg[:], cfg0[:], mybir.ActivationFunctionType.Exp, accum_out=zs[:]
    )
    rz = rp.tile([B, 1], _f32, tag="rz")
    nc.vector.reciprocal(rz[:], zs[:])
    cfgn = rp.tile([B, E], _f32, tag="cfgn")
    nc.vector.tensor_scalar_mul(cfgn[:], ecfg[:], rz[:])
    cfgf = rp.tile([B, E], _f32, tag="cfgf")
    nc.vector.scalar_tensor_tensor(
        cfgf[:], cfgn[:], SPARSE_EPS, cfgn[:], op0=_alu.is_ge, op1=_alu.mult
    )

    # transpose to [E, B], replicated into both partition halves so the
    # row-packed matmuls can source weights at array rows 0-63 and 64-127
    ident = rp.tile([B, B], _f32, tag="ident")
    masks.make_identity(nc, ident[:])
    psT = pp.tile([B, B], _f32, tag="ps")
    nc.tensor.matmul(psT[0:E, :], cfgf[:], ident[:], start=True, stop=True)
    nc.tensor.matmul(psT[E:2 * E, :], cfgf[:], ident[:], start=True, stop=True)
    cfgT2 = rp.tile([B, B], _f16, tag="cfgT2")
    nc.vector.tensor_copy(cfgT2[:], psT[:])
    return cfgT2


def _build():
    nc = bass.Bass(
        "TRN2", target_bir_lowering=False, debug=False, num_devices=N_CORES
    )
    cfg_ap = nc.dram_tensor("config", [B, E], _f32, kind="ExternalInput").ap()
    ks_ap = nc.dram_tensor(
        "kslice", [NT, 2 * E, 2 * E + (ROWS_IN // 2) * CH], _f16,
        kind="ExternalInput",
    ).ap()
    out_ap = nc.dram_tensor(
        "out", [D1_SH // ROWS_OUT, B, ROWS_OUT * CH], _f16,
        kind="ExternalOutput",
    ).ap()

    with tile.TileContext(nc) as tc:
        with tc.tile_pool(name="route", bufs=1) as rp, \
             tc.tile_pool(name="inp", bufs=4) as ip, \
             tc.tile_pool(name="outp", bufs=7) as op_, \
             tc.tile_pool(name="ps", bufs=8, space="PSUM") as pp:
            # the f32 config is bitcast-embedded as 128 f16 columns at
            # the head of every table tile, so there is NO separate config
            # DMA: the table stream owns the SP ring from preamble-end (a
            # 32 KB config DMA ahead of it costs ~6 us of ring
            # serialization; on the ACT ring it starves ~12 us behind the
            # table descriptor trains; SWDGE takes ~12 us to spin up).
            kts = []
            for t in range(2):
                kt = ip.tile(
                    [2 * E, 2 * E + (ROWS_IN // 2) * CH], _f16, tag="kt"
                )
                if t == 0:
                    # split tile 0 into a small head DMA (config + first
                    # chunk-pair, ~560 KB) and the remainder: the routing
                    # chain unblocks at ~5 us instead of waiting for the
                    # full 2 MB, so the matmul pipeline starts ~10 us
                    # earlier and the input stream never starves behind
                    # a compute crunch.  (Range-based deps: c=0 matmuls
                    # only wait on the head DMA.)
                    hd = 2 * E + CH
                    nc.sync.dma_start(kt[:, 0:hd], ks_ap[t][:, 0:hd])
                    nc.sync.dma_start(kt[:, hd:], ks_ap[t][:, hd:])
                else:
                    nc.sync.dma_start(kt[:], ks_ap[t])
                kts.append(kt)
            cfgin = kts[0][:, 0:2 * E].bitcast(_f32)
            cfgT2 = _routing_weights(nc, rp, pp, cfgin)
            n_out = 0
            for t in range(NT):
                if t < 2:
                    kt = kts[t]
                else:
                    kt = ip.tile(
                        [2 * E, 2 * E + (ROWS_IN // 2) * CH], _f16, tag="kt"
                    )
                    if t == NT - 1:
                        # split the last tile too: its first half's
                        # matmuls start ~3 us earlier, shortening the
                        # output-only drain at the end
                        md = 2 * E + 2 * CH
                        nc.sync.dma_start(kt[:, 0:md], ks_ap[t][:, 0:md])
                        nc.sync.dma_start(kt[:, md:], ks_ap[t][:, md:])
                    else:
                        nc.sync.dma_start(kt[:], ks_ap[t])
                for half in range(ROWS_IN // ROWS_OUT):
                    # output tile covers D1-rows 8t+4*half .. +3, as
                    # ROWS_OUT col-blocks of CH; input pair c (cols c*CH)
                    # holds rows (8t+2c, 8t+2c+1) in partition halves
                    ot = op_.tile([B, ROWS_OUT * CH], _f16, tag="ot")
                    for cc in range(ROWS_OUT // 2):
                        c = half * (ROWS_OUT // 2) + cc
                        for j in range(N_MM):
                            js_in = slice(2 * E + c * CH + j * MM_N,
                                          2 * E + c * CH + (j + 1) * MM_N)
                            jsA = slice(2 * cc * CH + j * MM_N,
                                        2 * cc * CH + (j + 1) * MM_N)
                            jsB = slice((2 * cc + 1) * CH + j * MM_N,
                                        (2 * cc + 1) * CH + (j + 1) * MM_N)
                            psA = pp.tile([B, MM_N], _f32, tag="ps")
                            nc.tensor.matmul(
                                psA[:], cfgT2[0:E, :], kt[0:E, js_in],
                                start=True, stop=True,
                            )
                            psB = pp.tile([B, MM_N], _f32, tag="ps")
                            nc.tensor.matmul(
                                psB[:], cfgT2[E:2 * E, :], kt[E:2 * E, js_in],
                                start=True, stop=True,
                            )
                            if j % 2 == 0:
                                nc.vector.tensor_copy(ot[:, jsA], psA[:])
                                nc.scalar.copy(ot[:, jsB], psB[:])
                            else:
                                nc.scalar.copy(ot[:, jsA], psA[:])
                                nc.vector.tensor_copy(ot[:, jsB], psB[:])
                    # alternate output DMAs across both HWDGE rings so the
                    # write stream can saturate HBM during the drain phase
                    eng = nc.scalar if n_out % 2 == 0 else nc.sync
                    n_out += 1
                    eng.dma_start(out_ap[2 * t + half], ot[:])
    _split_multi_waits(nc)
    return nc


_NC_CACHE = None


def _get_nc():
    global _NC_CACHE
    if _NC_CACHE is None:
        _NC_CACHE = _build()
    return _NC_CACHE


def kernel(config, kernel):
    global LAST_RESULT
    config = np.ascontiguousarray(np.asarray(config, dtype=np.float32))
    ktab = np.asarray(kernel, dtype=np.float32).reshape(E, D1, D2)

    in_maps = []
    for c in range(N_CORES):
        # this core's D1 rows, chunk-major [D1_SH, E, D2] fp16, regrouped
        # into 8 tiles [128, 4*D2]: partition i = (h*64+e) holds expert e
        # of D1-row 8t+2c2+h at free columns c2*D2..c2*D2+D2.
        ksl = ktab[:, c * D1_SH:(c + 1) * D1_SH, :].transpose(1, 0, 2)
        ksl = ksl.reshape(NT, ROWS_IN // 2, 2, E, D2).transpose(0, 2, 3, 1, 4)
        ksl = np.asarray(ksl, dtype=np.float16).reshape(
            NT, 2 * E, (ROWS_IN // 2) * D2
        )
        cfg16 = np.broadcast_to(
            config.view(np.float16)[None], (NT, 2 * E, 2 * E)
        )
        ksl = np.ascontiguousarray(np.concatenate([cfg16, ksl], axis=2))
        in_maps.append({"config": config, "kslice": ksl})

    nc = _get_nc()
    res = bass_utils.run_bass_kernel_spmd(
        nc,
        in_maps,
        list(range(N_CORES)),
        trace=_TRACE,
        **_TRACE_KWARGS,
    )
    LAST_RESULT = res

    out = np.empty((B, D1, D2), dtype=np.float32)
    for c in range(N_CORES):
        # out DRAM [16, B, 4*D2] fp16: row r holds D1-rows 4r..4r+3 as
        # 4 col-blocks of D2
        o = res.results[c]["out"].reshape(D1_SH // ROWS_OUT, B, ROWS_OUT, D2)
        o = o.transpose(0, 2, 1, 3).reshape(D1_SH, B, D2)
        out[:, c * D1_SH:(c + 1) * D1_SH, :] = o.transpose(1, 0, 2)
    return out
